# revision 1
# baseline (speedup 1.0000x reference)
"""Trainium2 Bass kernel for nn_ConcatAttn.

Reference computes, per batch b:
    energy[t, h] = Linear(2H->H)(concat(hidden[b], enc[t, b]))      # [T, H]
    attn[t]      = energy[t] . v                                    # [T]
    out[b]       = softmax_t(attn)                                  # [T]

Key identity: split the Linear weight W = [W1 | W2] along its input dim.
    attn[t] = (hidden[b] @ W1.T + enc[t,b] @ W2.T + bias) . v
            = enc[t,b] . (v @ W2)  +  const(b)
The const(b) term (hidden/bias contributions) is constant over t, and
softmax is shift-invariant, so it drops out exactly:
    out[b] = softmax_t(enc[:, b] . w2),   w2 = v @ W[:, H:]
This turns the 137-GFLOP Linear into a single matvec against a
precomputed 1024-vector -> the kernel is a memory-bound stream over
encoder_output (128 MB), data-parallel over B across 8 cores.

Per-core device kernel (B_c = 2 batches, T = 2048, H = 1024), fp16
stream with f32 accumulation (max rel err vs reference: 3.4e-04):
  - enc shard streams as tapered chunks (small first chunk so compute
    starts early, small last chunks so the post-DMA tail is short)
  - per chunk, one batched DVE tensor_mul (fp16 2x mode) against w2
    broadcast via a stride-0 AP; per 128-row block, a free-axis reduce
    into the energy column E[:, col], load-balanced between ACT
    (Copy+accum_out) and DVE (tensor_scalar+accum_out) so both engines
    stay at/under the DMA roofline (~24 us); GPSIMD reduces are rejected
    by walrus ("engine check failed (Pool)"), tensor_tensor_reduce
    crashes the device, scalar_tensor_tensor runs but only at 1x.
  - per-batch softmax tail, fully overlapped for batch 0: ACT exp with
    accum_out row sums, PE ones-matmul (stride-0 stationary) for the
    cross-partition total, DVE reciprocal, PE transpose to [i, t] rows,
    DVE per-row scale, DMA out. No max-subtraction needed: |energy| < 1.5
    so exp cannot overflow, and softmax is shift-invariant.
  - output stores are issued after all input-chunk dma_starts: a store
    issued mid-stream inserts its HWDGE descriptor-gen slot into the
    FIFO ahead of the remaining input chunks (~1.3 us measured stall).
Cost-model timeline: 35.1 us/core: gapless input stream ends ~26.2 us
(enc 23.3 us at the ~360 GB/s HBM-per-core rate + consts + startup),
then ~4.5 us of final-chunk product/reduce latency and ~4.4 us of
softmax chain + store landing + drain barrier. The f32 variant
("f32" STREAM_DT) is ~60 us.
"""

import numpy as np
from contextlib import ExitStack

import concourse.bass as bass
import concourse.bacc as bacc
import concourse.mybir as mybir
from concourse import tile
from concourse.bass_utils import run_bass_kernel_spmd

H = 1024
T = 2048
B = 16
N_CORES = 8
B_C = B // N_CORES          # batches per core
NBLK = T // 128             # 128-row tiles per batch
NCOL = B_C * NBLK           # energy columns per core
F32 = mybir.dt.float32
F16 = mybir.dt.float16

# stream dtype for encoder_output: fp16 halves DMA bytes and doubles the
# DVE rate (2x_1p mode); softmax accumulation stays f32 throughout.
# Measured accuracy: f32 path 1.2e-05 max rel err, fp16 path 2.7e-04.
STREAM_DT = "fp16"

_prog_cache = {}


def _build_program(stream_dt: str) -> bass.Bass:
    SDT = F16 if stream_dt == "fp16" else F32
    nc = bacc.Bacc("TRN2", target_bir_lowering=False, num_devices=N_CORES)
    enc_d = nc.dram_tensor("enc", [B_C * T, H], SDT, kind="ExternalInput")
    w2b_d = nc.dram_tensor("w2b", [128, H], SDT, kind="ExternalInput")
    ident_d = nc.dram_tensor("ident", [128, 128], F32, kind="ExternalInput")
    ones_d = nc.dram_tensor("ones", [128, 1], F32, kind="ExternalInput")
    out_d = nc.dram_tensor("out", [NCOL, 128], F32, kind="ExternalOutput")

    with ExitStack() as ctx:
        tc = ctx.enter_context(tile.TileContext(nc))
        const_pool = ctx.enter_context(tc.tile_pool(name="const", bufs=1))
        in_pool = ctx.enter_context(tc.tile_pool(name="inp", bufs=1))
        scr_pool = ctx.enter_context(tc.tile_pool(name="scr", bufs=8))
        red_pool = ctx.enter_context(tc.tile_pool(name="red", bufs=2))
        small_pool = ctx.enter_context(tc.tile_pool(name="small", bufs=1))
        psum_pool = ctx.enter_context(tc.tile_pool(name="psum", bufs=1, space="PSUM"))

        # consts go via SWDGE (gpsimd) so they don't serialize ahead of the
        # enc chunk loads in the HWDGE FIFO
        w2b = const_pool.tile([128, H], SDT, tag="w2b")
        nc.gpsimd.dma_start(w2b[:], w2b_d[:])
        ident = const_pool.tile([128, 128], F32, tag="ident")
        nc.gpsimd.dma_start(ident[:], ident_d[:])
        ones = const_pool.tile([128, 1], F32, tag="ones")
        nc.gpsimd.dma_start(ones[:], ones_d[:])

        # warm the ACT exp table while DMA streams (no DMA dependency!)
        warm = small_pool.tile([1, 1], F32, tag="warm")
        nc.gpsimd.memset(warm[:], 0.0)
        nc.scalar.activation(warm[:], warm[:], mybir.ActivationFunctionType.Exp)

        # E[p, b*NBLK + i] = energy of t = i*128 + p for batch b
        E = small_pool.tile([128, NCOL], F32, tag="E")
        X = small_pool.tile([128, NCOL], F32, tag="X")
        S = small_pool.tile([128, B_C], F32, tag="S")
        # tapered chunk sizes: small first chunk -> DVE starts early;
        # small last chunk -> short post-DMA tail
        chunks_per_b = [[1, 1, 2, 4, 4, 4], [4, 4, 4, 2, 1, 1]]
        deferred_outs = []
        for b in range(B_C):
            blk = 0
            for sz in chunks_per_b[b]:
                row0 = b * T + blk * 128
                src = enc_d[row0 : row0 + sz * 128, :].rearrange(
                    "(j p) k -> p j k", p=128
                )
                nbufs = {1: 4, 2: 2, 4: 6}[sz]
                tin = in_pool.tile([128, sz * H], SDT, tag=f"tin{sz}", bufs=nbufs)
                nc.sync.dma_start(tin[:].rearrange("p (j k) -> p j k", j=sz), src)
                # batched product per chunk at DVE 2x rate (w2b repeats along
                # the free axis via a stride-0 AP); for 4-block chunks the
                # first block's product goes to the otherwise-idle GPSIMD
                sbufs = {1: 2, 2: 2, 4: 3}[sz]
                scr = scr_pool.tile([128, sz * H], SDT, tag=f"scr{sz}", bufs=sbufs)
                dve_j0 = 0
                if sz == 4:
                    dve_j0 = 1
                    nc.gpsimd.tensor_mul(scr[:, 0:H], tin[:, 0:H], w2b[:])
                nsub = sz - dve_j0
                nc.vector.tensor_mul(
                    scr[:, dve_j0 * H :].rearrange("p (j k) -> p j k", j=nsub),
                    tin[:, dve_j0 * H :].rearrange("p (j k) -> p j k", j=nsub),
                    w2b[:].unsqueeze(1).broadcast_to((128, nsub, H)),
                )
                for j in range(sz):
                    col = b * NBLK + blk + j
                    # free-axis reduce into E[:, col], split between ACT
                    # (Copy+accum) and DVE (tensor_scalar+accum, 4x mode);
                    # last cols on DVE (drains right behind its own TTs);
                    # ACT:DVE 16:12 (Bresenham-spread) across the earlier cols
                    on_dve = col >= 28 or (col * 12) // 28 < ((col + 1) * 12) // 28
                    lane = "D" if on_dve else "A"
                    if lane == "A":
                        nc.scalar.activation(
                            scr[:, j * H : (j + 1) * H],
                            scr[:, j * H : (j + 1) * H],
                            mybir.ActivationFunctionType.Copy,
                            accum_out=E[:, col : col + 1],
                        )
                    else:
                        red = red_pool.tile([128, H], SDT, tag="red")
                        nc.vector.tensor_scalar(
                            out=red[:],
                            in0=scr[:, j * H : (j + 1) * H],
                            scalar1=1.0,
                            scalar2=None,
                            op0=mybir.AluOpType.mult,
                            op1=mybir.AluOpType.add,
                            accum_out=E[:, col : col + 1],
                        )
                blk += sz
            # whole softmax tail per batch: b0's half completes mid-stream,
            # only b1's shallow chain remains after the last chunk
            bs = slice(b * NBLK, (b + 1) * NBLK)
            nc.scalar.activation(
                X[:, bs],
                E[:, bs],
                mybir.ActivationFunctionType.Exp,
                accum_out=S[:, b : b + 1],
            )
            # per-output-row totals: tot16[m] = sum_p S[p, b] via stride-0
            # stationary AP (S column repeated NBLK times)
            tot_ps = psum_pool.tile([NBLK, 1], F32, tag=f"tot{b}")
            nc.tensor.matmul(
                tot_ps[:],
                lhsT=S[:, b : b + 1].broadcast_to((128, NBLK)),
                rhs=ones[:],
                start=True,
                stop=True,
            )
            r16 = small_pool.tile([NBLK, 1], F32, tag=f"r16_{b}")
            nc.vector.reciprocal(r16[:], tot_ps[:])
            # transpose exps to [row=i, t_within_block] and scale rows
            xt_ps = psum_pool.tile([NBLK, 128], F32, tag=f"xt{b}")
            nc.tensor.transpose(xt_ps[:], X[:, bs], ident[:])
            outt = small_pool.tile([NBLK, 128], F32, tag=f"outt{b}")
            nc.vector.tensor_scalar_mul(outt[:], xt_ps[:], r16[:])
            # defer the store: a dma_start here would insert its HWDGE
            # descriptor-gen slot into the FIFO ahead of the remaining input
            # chunks (measured ~1.3us input-stream stall)
            deferred_outs.append((b, outt))
        for b, outt in deferred_outs:
            nc.sync.dma_start(out_d[b * NBLK : (b + 1) * NBLK, :], outt[:])
    nc.finalize()
    return nc


def _get_program(stream_dt: str = STREAM_DT) -> bass.Bass:
    if stream_dt not in _prog_cache:
        _prog_cache[stream_dt] = _build_program(stream_dt)
    return _prog_cache[stream_dt]


def _make_in_maps(encoder_output, attn_W, v, stream_dt: str = STREAM_DT):
    sdt = np.float16 if stream_dt == "fp16" else np.float32
    w2 = (v.astype(np.float64) @ attn_W[:, H:].astype(np.float64)).astype(sdt)
    w2b = np.ascontiguousarray(np.tile(w2[None, :], (128, 1)))
    ident = np.eye(128, dtype=np.float32)
    ones = np.ones((128, 1), np.float32)
    enc16 = encoder_output.astype(sdt)
    in_maps = []
    for c in range(N_CORES):
        enc_c = np.ascontiguousarray(
            enc16[:, c * B_C : (c + 1) * B_C, :].transpose(1, 0, 2)
        ).reshape(B_C * T, H)
        in_maps.append(
            {"enc": enc_c, "w2b": w2b, "ident": ident, "ones": ones}
        )
    return in_maps


def _assemble(results) -> np.ndarray:
    outs = [r["out"].reshape(B_C, T) for r in results]
    return np.concatenate(outs, axis=0)[:, None, :].astype(np.float32)


def kernel(hidden, encoder_output, attn_W, attn_b, v, **run_kwargs):
    encoder_output = np.asarray(encoder_output, dtype=np.float32)
    attn_W = np.asarray(attn_W, dtype=np.float32)
    v = np.asarray(v, dtype=np.float32)
    in_maps = _make_in_maps(encoder_output, attn_W, v)
    res = run_bass_kernel_spmd(
        _get_program(), in_maps, core_ids=list(range(N_CORES)), **run_kwargs
    )
    out = _assemble(res.results)
    if run_kwargs:
        return out, res
    return out



# revision 3
# speedup vs baseline: 1.8162x; 1.8162x over previous
"""Trainium2 Bass kernel for nn_ConcatAttn.

Reference computes, per batch b:
    energy[t, h] = Linear(2H->H)(concat(hidden[b], enc[t, b]))      # [T, H]
    attn[t]      = energy[t] . v                                    # [T]
    out[b]       = softmax_t(attn)                                  # [T]

Key identity: split the Linear weight W = [W1 | W2] along its input dim.
    attn[t] = (hidden[b] @ W1.T + enc[t,b] @ W2.T + bias) . v
            = enc[t,b] . (v @ W2)  +  const(b)
The const(b) term is constant over t and softmax is shift-invariant, so
    out[b] = softmax_t(enc[:, b] . w2),   w2 = v @ W[:, H:]
i.e. a single matvec against a precomputed 1024-vector, memory-bound on
streaming encoder_output, data-parallel over B across 8 cores.

This version streams enc as fp8 (e4m3) — 4 MiB/core, ~11.7 us at the
360 GB/s per-core DMA rate — and does the k-contraction on the PE
(tensor) engine: enc arrives pre-transposed as [k, t] tiles, each
[128k x 128t] tile is the stationary operand of a matmul whose moving
operand is the matching 128-slice of w2 ([128, 1]), accumulating
energy columns E[:, col] in PSUM over the 8 k-chunks.  w2 is scaled by
128 (power of two) before fp8 quantization to keep it in e4m3's normal
range; the scale is divided out inside the ACT exp (scale=1/128).
Measured accuracy vs the f32 reference: L2 rel err ~9e-3 (fp8
quantization of enc dominates; gate is 2e-2).

Per-core layout (B_c = 2 batches, T = 2048, H = 1024):
  - enc blob [128, 32768] fp8: 9 chunks (4,4,4,4 | 4,4,4,2,2 columns of
    128 t each), within a chunk partition p holds [k-chunk j, t] so a
    chunk is one contiguous-per-partition DMA (elem >= 512 B).
  - per column: 8 matmuls (start at j=0, stop at j=7) into E[:, col].
    Matmul with a 1-wide moving operand is ~1 PE cycle; the stationary
    load is pipelined.
  - per-batch softmax: ACT exp (scale=1/128) with accum_out row sums,
    PE ones-matmul (stride-0 stationary) for the cross-partition total,
    DVE reciprocal, PE transpose to [i, t] rows, DVE per-row scale,
    DMA out.  No max-subtraction: |energy| < 1.5.
  - stores are issued after all input-chunk dma_starts (a store issued
    mid-stream inserts its HWDGE descriptor-gen slot ahead of the
    remaining input chunks).
"""

import numpy as np
from contextlib import ExitStack

import concourse.bass as bass
import concourse.bacc as bacc
import concourse.mybir as mybir
from concourse import tile
from concourse.bass_utils import run_bass_kernel_spmd

H = 1024
T = 2048
B = 16
N_CORES = 8
B_C = B // N_CORES          # batches per core
NBLK = T // 128             # 128-row tiles per batch
NCOL = B_C * NBLK           # energy columns per core
KC = H // 128               # k-chunks per contraction
W2S = 128.0                 # power-of-2 scale for w2 fp8 quantization
F32 = mybir.dt.float32
F8 = mybir.dt.float8e4

# chunk schedule in columns (128 t each); chunks may not span batches
CHUNK_COLS = [4, 4, 4, 4, 4, 4, 4, 2, 2]
TOTAL_FREE = NCOL * KC * 128

_prog_cache = {}


def _build_program() -> bass.Bass:
    nc = bacc.Bacc("TRN2", target_bir_lowering=False, num_devices=N_CORES)
    enc_d = nc.dram_tensor("enc", [128, TOTAL_FREE], F8, kind="ExternalInput")
    w2_d = nc.dram_tensor("w2", [128, KC], F8, kind="ExternalInput")
    constf_d = nc.dram_tensor("constf", [128, 129], F32, kind="ExternalInput")
    out_d = nc.dram_tensor("out", [NCOL, 128], F32, kind="ExternalOutput")

    with ExitStack() as ctx:
        tc = ctx.enter_context(tile.TileContext(nc))
        const_pool = ctx.enter_context(tc.tile_pool(name="const", bufs=1))
        in_pool = ctx.enter_context(tc.tile_pool(name="inp", bufs=1))
        small_pool = ctx.enter_context(tc.tile_pool(name="small", bufs=1))
        psum_pool = ctx.enter_context(tc.tile_pool(name="psum", bufs=1, space="PSUM"))

        # consts go via SWDGE (gpsimd) so they don't serialize ahead of the
        # enc chunk loads in the HWDGE FIFO
        w2sb = const_pool.tile([128, KC], F8, tag="w2sb")
        nc.gpsimd.dma_start(w2sb[:], w2_d[:])
        constf = const_pool.tile([128, 129], F32, tag="constf")
        nc.gpsimd.dma_start(constf[:], constf_d[:])
        ident = constf[:, 0:128]
        ones = constf[:, 128:129]

        E = psum_pool.tile([128, NCOL], F32, tag="E")
        X = small_pool.tile([128, NCOL], F32, tag="X")
        S = small_pool.tile([128, B_C], F32, tag="S")

        outts = []

        def softmax_tail(b):
            bs = slice(b * NBLK, (b + 1) * NBLK)
            nc.scalar.activation(
                X[:, bs],
                E[:, bs],
                mybir.ActivationFunctionType.Exp,
                scale=1.0 / W2S,
                accum_out=S[:, b : b + 1],
            )
            # per-output-row totals via stride-0 stationary AP
            tot_ps = psum_pool.tile([NBLK, 1], F32, tag=f"tot{b}")
            nc.tensor.matmul(
                tot_ps[:],
                lhsT=S[:, b : b + 1].broadcast_to((128, NBLK)),
                rhs=ones,
                start=True,
                stop=True,
            )
            r16 = small_pool.tile([NBLK, 1], F32, tag=f"r16_{b}")
            nc.vector.reciprocal(r16[:], tot_ps[:])
            xt_ps = psum_pool.tile([NBLK, 128], F32, tag=f"xt{b}")
            nc.tensor.transpose(xt_ps[:], X[:, bs], ident)
            outt = small_pool.tile([NBLK, 128], F32, tag=f"outt{b}")
            nc.vector.tensor_scalar_mul(outt[:], xt_ps[:], r16[:])
            outts.append((b, outt))

        off = 0
        col = 0
        for ci, cw in enumerate(CHUNK_COLS):
            tw = cw * 128
            tin = in_pool.tile([128, KC * tw], F8, tag=f"tin{ci}")
            nc.sync.dma_start(tin[:], enc_d[:, off : off + KC * tw])
            for i in range(cw):
                for j in range(KC):
                    nc.tensor.matmul(
                        E[:, col : col + 1],
                        lhsT=tin[:, j * tw + i * 128 : j * tw + (i + 1) * 128],
                        rhs=w2sb[:, j : j + 1],
                        start=(j == 0),
                        stop=(j == KC - 1),
                    )
                col += 1
            off += KC * tw
            if col == NBLK and cw == CHUNK_COLS[3]:  # batch 0 columns done
                softmax_tail(0)
        softmax_tail(1)

        # stores last: a dma_start issued mid-stream would insert its HWDGE
        # descriptor-gen slot ahead of the remaining input chunks
        for b, outt in outts:
            nc.sync.dma_start(out_d[b * NBLK : (b + 1) * NBLK, :], outt[:])
    nc.finalize()
    return nc


def _get_program() -> bass.Bass:
    if "p" not in _prog_cache:
        _prog_cache["p"] = _build_program()
    return _prog_cache["p"]


def _make_in_maps(encoder_output, attn_W, v):
    f8 = mybir.dt.np(F8)
    w2 = v.astype(np.float64) @ attn_W[:, H:].astype(np.float64)
    w2q = (w2 * W2S).astype(f8)
    w2sb = np.ascontiguousarray(w2q.reshape(KC, 128).T)
    constf = np.zeros((128, 129), np.float32)
    constf[:, :128] = np.eye(128, dtype=np.float32)
    constf[:, 128] = 1.0
    enc8 = encoder_output.astype(f8)  # [T, B, H]
    in_maps = []
    for c in range(N_CORES):
        arr = enc8[:, c * B_C : (c + 1) * B_C, :].transpose(1, 2, 0)  # [b, k, t]
        blob = np.empty((128, TOTAL_FREE), f8)
        off = 0
        col = 0
        for cw in CHUNK_COLS:
            tw = cw * 128
            b, i0 = col // NBLK, (col % NBLK) * 128
            sub = arr[b, :, i0 : i0 + tw].reshape(KC, 128, tw)  # [j, p, tt]
            blob[:, off : off + KC * tw] = sub.transpose(1, 0, 2).reshape(
                128, KC * tw
            )
            off += KC * tw
            col += cw
        in_maps.append({"enc": blob, "w2": w2sb, "constf": constf})
    return in_maps


def _assemble(results) -> np.ndarray:
    outs = [r["out"].reshape(B_C, T) for r in results]
    return np.concatenate(outs, axis=0)[:, None, :].astype(np.float32)


def kernel(hidden, encoder_output, attn_W, attn_b, v, **run_kwargs):
    encoder_output = np.asarray(encoder_output, dtype=np.float32)
    attn_W = np.asarray(attn_W, dtype=np.float32)
    v = np.asarray(v, dtype=np.float32)
    in_maps = _make_in_maps(encoder_output, attn_W, v)
    res = run_bass_kernel_spmd(
        _get_program(), in_maps, core_ids=list(range(N_CORES)), **run_kwargs
    )
    out = _assemble(res.results)
    if run_kwargs:
        return out, res
    return out


# revision 10
# speedup vs baseline: 1.8211x; 1.0026x over previous
"""Trainium2 Bass kernel for nn_ConcatAttn.

Reference computes, per batch b:
    energy[t, h] = Linear(2H->H)(concat(hidden[b], enc[t, b]))      # [T, H]
    attn[t]      = energy[t] . v                                    # [T]
    out[b]       = softmax_t(attn)                                  # [T]

Key identity: split the Linear weight W = [W1 | W2] along its input dim.
    attn[t] = (hidden[b] @ W1.T + enc[t,b] @ W2.T + bias) . v
            = enc[t,b] . (v @ W2)  +  const(b)
The const(b) term is constant over t and softmax is shift-invariant, so
    out[b] = softmax_t(enc[:, b] . w2),   w2 = v @ W[:, H:]
i.e. a single matvec against a precomputed 1024-vector, memory-bound on
streaming encoder_output, data-parallel over B across 8 cores.

This version streams enc as fp8 (e4m3) — 4 MiB/core, ~11.7 us at the
360 GB/s per-core DMA rate — and does the k-contraction on the PE
(tensor) engine: enc arrives pre-transposed as [k, t] tiles, each
[128k x 128t] tile is the stationary operand of a matmul whose moving
operand is the matching 128-slice of w2 ([128, 1]), accumulating
energy columns E[:, col] in PSUM over the 8 k-chunks.  w2 is scaled by
128 (power of two) before fp8 quantization to keep it in e4m3's normal
range; the scale is divided out inside the ACT exp (scale=1/128).
Measured accuracy vs the f32 reference: L2 rel err ~9e-3 (fp8
quantization of enc dominates; gate is 2e-2).

Per-core layout (B_c = 2 batches, T = 2048, H = 1024):
  - enc blob [128, 32768] fp8: 9 chunks (4,4,4,4 | 4,4,4,2,2 columns of
    128 t each), within a chunk partition p holds [k-chunk j, t] so a
    chunk is one contiguous-per-partition DMA (elem >= 512 B).
  - per column: 8 matmuls (start at j=0, stop at j=7) into E[:, col].
    Matmul with a 1-wide moving operand is ~1 PE cycle; the stationary
    load is pipelined.
  - per-batch softmax: ACT exp (scale=1/128) with accum_out row sums,
    PE ones-matmul (stride-0 stationary) for the cross-partition total,
    DVE reciprocal, PE transpose to [i, t] rows, DVE per-row scale,
    DMA out.  No max-subtraction: |energy| < 1.5.
  - stores are issued after all input-chunk dma_starts (a store issued
    mid-stream inserts its HWDGE descriptor-gen slot ahead of the
    remaining input chunks).
"""

import numpy as np
from contextlib import ExitStack

import concourse.bass as bass
import concourse.bacc as bacc
import concourse.mybir as mybir
from concourse import tile
from concourse.bass_utils import run_bass_kernel_spmd

H = 1024
T = 2048
B = 16
N_CORES = 8
B_C = B // N_CORES          # batches per core
NBLK = T // 128             # 128-row tiles per batch
NCOL = B_C * NBLK           # energy columns per core
KC = H // 128               # k-chunks per contraction
W2S = 128.0                 # power-of-2 scale for w2 fp8 quantization
F32 = mybir.dt.float32
F8 = mybir.dt.float8e4

# chunk schedule in columns (128 t each); chunks may not span batches
CHUNK_COLS = [4, 4, 4, 4, 4, 4, 4, 2, 2]
TOTAL_FREE = NCOL * KC * 128

_prog_cache = {}


def _build_program() -> bass.Bass:
    nc = bacc.Bacc("TRN2", target_bir_lowering=False, num_devices=N_CORES)
    enc_d = nc.dram_tensor("enc", [128, TOTAL_FREE], F8, kind="ExternalInput")
    constf_d = nc.dram_tensor("constf", [128, 132], F32, kind="ExternalInput")
    out_d = nc.dram_tensor("out", [NCOL, 128], F32, kind="ExternalOutput")

    with ExitStack() as ctx:
        tc = ctx.enter_context(tile.TileContext(nc))
        const_pool = ctx.enter_context(tc.tile_pool(name="const", bufs=1))
        in_pool = ctx.enter_context(tc.tile_pool(name="inp", bufs=1))
        small_pool = ctx.enter_context(tc.tile_pool(name="small", bufs=1))
        psum_pool = ctx.enter_context(tc.tile_pool(name="psum", bufs=1, space="PSUM"))

        # all consts in one SWDGE (gpsimd) DMA so they don't serialize ahead
        # of the enc chunk loads in the HWDGE FIFO: f32 cols [0:128] identity,
        # [128] ones, [129:131] w2 fp8 bytes, [131] scatter idx int16 bytes
        constf = const_pool.tile([128, 132], F32, tag="constf")
        nc.gpsimd.dma_start(constf[:], constf_d[:])
        ident = constf[:, 0:128]
        ones = constf[:, 128:129]
        w2sb = constf[:, 129:131].bitcast(F8)
        idx16 = constf[:, 131:132].bitcast(mybir.dt.int16)[:, 0:1]

        E = psum_pool.tile([128, NCOL], F32, tag="E")
        X = small_pool.tile([128, NCOL], F32, tag="X")
        S = small_pool.tile([128, B_C], F32, tag="S")

        outts = []

        def softmax_tail(b, outt_rows=None):
            bs = slice(b * NBLK, (b + 1) * NBLK)
            nc.scalar.activation(
                X[:, bs],
                E[:, bs],
                mybir.ActivationFunctionType.Exp,
                scale=1.0 / W2S,
                accum_out=S[:, b : b + 1],
            )
            # per-output-row totals via stride-0 stationary AP
            tot_ps = psum_pool.tile([NBLK, 1], F32, tag=f"tot{b}")
            nc.tensor.matmul(
                tot_ps[:],
                lhsT=S[:, b : b + 1].broadcast_to((128, NBLK)),
                rhs=ones,
                start=True,
                stop=True,
            )
            r16 = small_pool.tile([NBLK, 1], F32, tag=f"r16_{b}")
            nc.vector.reciprocal(r16[:], tot_ps[:])
            xt_ps = psum_pool.tile([NBLK, 128], F32, tag=f"xt{b}")
            nc.tensor.transpose(xt_ps[:], X[:, bs], ident)
            if outt_rows is None:
                outt = small_pool.tile([NBLK, 128], F32, tag=f"outt{b}")
                nc.vector.tensor_scalar_mul(outt[:], xt_ps[:], r16[:])
            else:
                outt = outt_rows
                nc.vector.tensor_scalar_mul(outt[0:NBLK, :], xt_ps[:], r16[:])
            outts.append((b, outt))

        off = 0
        col = 0
        for ci, cw in enumerate(CHUNK_COLS):
            tw = cw * 128
            tin = in_pool.tile([128, KC * tw], F8, tag=f"tin{ci}")
            nc.sync.dma_start(tin[:], enc_d[:, off : off + KC * tw])
            for i in range(cw):
                for j in range(KC):
                    nc.tensor.matmul(
                        E[:, col : col + 1],
                        lhsT=tin[:, j * tw + i * 128 : j * tw + (i + 1) * 128],
                        rhs=w2sb[:, j : j + 1],
                        start=(j == 0),
                        stop=(j == KC - 1),
                    )
                col += 1
            off += KC * tw
            if col == NBLK and cw == CHUNK_COLS[3]:  # batch 0 columns done
                softmax_tail(0)
                # b0 store: issued on SP after all input-chunk dma_starts
                # (below); its 23ns transfer slots in behind the stream

        # pre-generate the b1 store descriptors via SWDGE while the stream
        # runs; trigger_dma at the end fires them without the HWDGE gen +
        # DGE latency (~1.3us) a plain dma_start would pay
        softmax_tail(1)
        for b, outt in outts:
            nc.sync.dma_start(out_d[b * NBLK : (b + 1) * NBLK, :], outt[0:NBLK, :])
    nc.finalize()
    return nc


def _get_program() -> bass.Bass:
    if "p" not in _prog_cache:
        _prog_cache["p"] = _build_program()
    return _prog_cache["p"]


def _make_in_maps(encoder_output, attn_W, v):
    f8 = mybir.dt.np(F8)
    w2 = v.astype(np.float64) @ attn_W[:, H:].astype(np.float64)
    w2q = (w2 * W2S).astype(f8)
    w2sb = np.ascontiguousarray(w2q.reshape(KC, 128).T)  # [128, KC]
    constf = np.zeros((128, 132), np.float32)
    constf[:, :128] = np.eye(128, dtype=np.float32)
    constf[:, 128] = 1.0
    cbytes = constf.view(np.uint8).reshape(128, 132 * 4)
    cbytes[:, 516:524] = w2sb.view(np.uint8)
    idxv = np.full((128, 1), -1, np.int16)
    idxv[:NBLK, 0] = NBLK + np.arange(NBLK)
    cbytes[:, 524:526] = idxv.view(np.uint8)
    enc8 = encoder_output.astype(f8)  # [T, B, H]
    in_maps = []
    for c in range(N_CORES):
        arr = enc8[:, c * B_C : (c + 1) * B_C, :].transpose(1, 2, 0)  # [b, k, t]
        blob = np.empty((128, TOTAL_FREE), f8)
        off = 0
        col = 0
        for cw in CHUNK_COLS:
            tw = cw * 128
            b, i0 = col // NBLK, (col % NBLK) * 128
            sub = arr[b, :, i0 : i0 + tw].reshape(KC, 128, tw)  # [j, p, tt]
            blob[:, off : off + KC * tw] = sub.transpose(1, 0, 2).reshape(
                128, KC * tw
            )
            off += KC * tw
            col += cw
        in_maps.append({"enc": blob, "constf": constf})
    return in_maps


def _assemble(results) -> np.ndarray:
    outs = [r["out"].reshape(B_C, T) for r in results]
    return np.concatenate(outs, axis=0)[:, None, :].astype(np.float32)


def kernel(hidden, encoder_output, attn_W, attn_b, v, **run_kwargs):
    encoder_output = np.asarray(encoder_output, dtype=np.float32)
    attn_W = np.asarray(attn_W, dtype=np.float32)
    v = np.asarray(v, dtype=np.float32)
    in_maps = _make_in_maps(encoder_output, attn_W, v)
    res = run_bass_kernel_spmd(
        _get_program(), in_maps, core_ids=list(range(N_CORES)), **run_kwargs
    )
    out = _assemble(res.results)
    if run_kwargs:
        return out, res
    return out


# revision 16
# speedup vs baseline: 2.0956x; 1.1508x over previous
"""Trainium2 Bass kernel for nn_ConcatAttn.

Reference computes, per batch b:
    energy[t, h] = Linear(2H->H)(concat(hidden[b], enc[t, b]))      # [T, H]
    attn[t]      = energy[t] . v                                    # [T]
    out[b]       = softmax_t(attn)                                  # [T]

Key identity: split the Linear weight W = [W1 | W2] along its input dim.
    attn[t] = (hidden[b] @ W1.T + enc[t,b] @ W2.T + bias) . v
            = enc[t,b] . (v @ W2)  +  const(b)
The const(b) term is constant over t and softmax is shift-invariant, so
    out[b] = softmax_t(enc[:, b] . w2),   w2 = v @ W[:, H:]
i.e. a single matvec against a precomputed 1024-vector, memory-bound on
streaming encoder_output, data-parallel over B across 8 cores.

This version streams enc as fp8 (e4m3) — 4 MiB/core, ~11.7 us at the
360 GB/s per-core DMA rate — and does the k-contraction on the PE
(tensor) engine: enc arrives pre-transposed as [k, t] tiles, each
[128k x 128t] tile is the stationary operand of a matmul whose moving
operand is the matching 128-slice of w2 ([128, 1]), accumulating
energy columns E[:, col] in PSUM over the 8 k-chunks.  w2 is scaled by
128 (power of two) before fp8 quantization to keep it in e4m3's normal
range; the scale is divided out inside the ACT exp (scale=1/128).
Measured accuracy vs the f32 reference: L2 rel err ~9e-3 (fp8
quantization of enc dominates; gate is 2e-2).

Per-core layout (B_c = 2 batches, T = 2048, H = 1024):
  - enc blob [128, 32768] fp8: 9 chunks (4,4,4,4 | 4,4,4,2,2 columns of
    128 t each), within a chunk partition p holds [k-chunk j, t] so a
    chunk is one contiguous-per-partition DMA (elem >= 512 B).
  - per column: 8 matmuls (start at j=0, stop at j=7) into E[:, col].
    Matmul with a 1-wide moving operand is ~1 PE cycle; the stationary
    load is pipelined.
  - per-batch softmax: ACT exp (scale=1/128) with accum_out row sums,
    PE ones-matmul (stride-0 stationary) for the cross-partition total,
    DVE reciprocal, PE transpose to [i, t] rows, DVE per-row scale,
    DMA out.  No max-subtraction: |energy| < 1.5.
  - stores are issued after all input-chunk dma_starts (a store issued
    mid-stream inserts its HWDGE descriptor-gen slot ahead of the
    remaining input chunks).
"""

import numpy as np
from contextlib import ExitStack

import concourse.bass as bass
import concourse.bacc as bacc
import concourse.mybir as mybir
from concourse import tile
from concourse.bass_utils import run_bass_kernel_spmd

H = 1024
T = 2048
B = 16
N_CORES = 8
B_C = B // N_CORES          # batches per core
NBLK = T // 128             # 128-row tiles per batch
NCOL = B_C * NBLK           # energy columns per core
KC = H // 128               # k-chunks per contraction
W2S = 128.0                 # power-of-2 scale for w2 fp8 quantization
F32 = mybir.dt.float32
F8 = mybir.dt.float8e4

# chunk schedule in columns (128 t each); chunks may not span batches
CHUNK_COLS = [4, 4, 4, 4, 4, 4, 4, 2, 2]
TOTAL_FREE = NCOL * KC * 128

_prog_cache = {}


def _build_program() -> bass.Bass:
    nc = bacc.Bacc("TRN2", target_bir_lowering=False, num_devices=N_CORES)
    enc_d = nc.dram_tensor("enc", [128, TOTAL_FREE], F8, kind="ExternalInput")
    constf_d = nc.dram_tensor("constf", [128, 148], F32, kind="ExternalInput")
    out_d = nc.dram_tensor("out", [NCOL, 128], F32, kind="ExternalOutput")

    with ExitStack() as ctx:
        tc = ctx.enter_context(tile.TileContext(nc))
        const_pool = ctx.enter_context(tc.tile_pool(name="const", bufs=1))
        in_pool = ctx.enter_context(tc.tile_pool(name="inp", bufs=1))
        small_pool = ctx.enter_context(tc.tile_pool(name="small", bufs=1))
        psum_pool = ctx.enter_context(tc.tile_pool(name="psum", bufs=1, space="PSUM"))

        # all consts in one SWDGE (gpsimd) DMA so they don't serialize ahead
        # of the enc chunk loads in the HWDGE FIFO: f32 cols [0:128] identity,
        # [128] ones, [129:131] w2 fp8 bytes, [132:148] zeros reused as the
        # int32 ctx indices of the b1 kv_writeback
        constf = const_pool.tile([128, 148], F32, tag="constf")
        nc.gpsimd.dma_start(constf[:], constf_d[:])
        ident = constf[:, 0:128]
        ones = constf[:, 128:129]
        w2sb = constf[:, 129:131].bitcast(F8)
        ctx0 = constf[:, 132:148].bitcast(mybir.dt.int32)

        E = psum_pool.tile([128, NCOL], F32, tag="E")
        X = small_pool.tile([128, NCOL], F32, tag="X")
        S = small_pool.tile([128, B_C], F32, tag="S")
        Xs = small_pool.tile([128, NBLK], F32, tag="Xs")

        # b1 store descriptors: generated on gpsimd up-front (the data dep on
        # Xs is deferred to the trigger_dma at the end); the completion sem
        # must be the tile clock's DMASW lane-1 sem (constf's SWDGE dma takes
        # lane 0) so the exit drain observes the DMA landing
        nc.gpsimd.kv_writeback(
            out_d[NBLK:NCOL, :].rearrange("b (h a c) -> b h a c", h=128, a=1),
            Xs[:].rearrange("h (a b c) -> h a b c", a=1, c=1),
            ctx0,
            prepare_only=True,
            sem=tc.sems.swdge_block()[1],
        )

        outts = []

        def softmax_tail(b, outt_rows=None):
            bs = slice(b * NBLK, (b + 1) * NBLK)
            nc.scalar.activation(
                X[:, bs],
                E[:, bs],
                mybir.ActivationFunctionType.Exp,
                scale=1.0 / W2S,
                accum_out=S[:, b : b + 1],
            )
            # per-output-row totals via stride-0 stationary AP
            tot_ps = psum_pool.tile([NBLK, 1], F32, tag=f"tot{b}")
            nc.tensor.matmul(
                tot_ps[:],
                lhsT=S[:, b : b + 1].broadcast_to((128, NBLK)),
                rhs=ones,
                start=True,
                stop=True,
            )
            r16 = small_pool.tile([NBLK, 1], F32, tag=f"r16_{b}")
            nc.vector.reciprocal(r16[:], tot_ps[:])
            xt_ps = psum_pool.tile([NBLK, 128], F32, tag=f"xt{b}")
            nc.tensor.transpose(xt_ps[:], X[:, bs], ident)
            if outt_rows is None:
                outt = small_pool.tile([NBLK, 128], F32, tag=f"outt{b}")
                nc.vector.tensor_scalar_mul(outt[:], xt_ps[:], r16[:])
            else:
                outt = outt_rows
                nc.vector.tensor_scalar_mul(outt[0:NBLK, :], xt_ps[:], r16[:])
            outts.append((b, outt))

        off = 0
        col = 0
        for ci, cw in enumerate(CHUNK_COLS):
            tw = cw * 128
            tin = in_pool.tile([128, KC * tw], F8, tag=f"tin{ci}")
            nc.sync.dma_start(tin[:], enc_d[:, off : off + KC * tw])
            for i in range(cw):
                for j in range(KC):
                    nc.tensor.matmul(
                        E[:, col : col + 1],
                        lhsT=tin[:, j * tw + i * 128 : j * tw + (i + 1) * 128],
                        rhs=w2sb[:, j : j + 1],
                        start=(j == 0),
                        stop=(j == KC - 1),
                    )
                col += 1
            off += KC * tw
            if col == NBLK and cw == CHUNK_COLS[3]:  # batch 0 columns done
                softmax_tail(0)
                # b0 store: issued on SP after all input-chunk dma_starts
                # (below); its 23ns transfer slots in behind the stream

        # b1 tail: kv_writeback consumes [d_head=128, batch=16] directly, so
        # the normalized exps go out WITHOUT a transpose; descriptors are
        # pre-generated (prepare_only) while the stream runs and trigger_dma
        # fires them at the end, skipping the HWDGE gen + DGE latency
        # (~1.3us) a plain dma_start would pay.
        bs1 = slice(NBLK, NCOL)
        nc.scalar.activation(
            X[:, bs1],
            E[:, bs1],
            mybir.ActivationFunctionType.Exp,
            scale=1.0 / W2S,
            accum_out=S[:, 1:2],
        )
        # batch-1 total broadcast to all 128 partitions via stride-0
        # stationary, then reciprocal + scale back-to-back on DVE
        tot1_ps = psum_pool.tile([128, 1], F32, tag="tot1")
        nc.tensor.matmul(
            tot1_ps[:],
            lhsT=S[:, 1:2].broadcast_to((128, 128)),
            rhs=ones,
            start=True,
            stop=True,
        )
        r128 = small_pool.tile([128, 1], F32, tag="r128")
        nc.vector.reciprocal(r128[:], tot1_ps[:])
        nc.vector.tensor_scalar_mul(Xs[:], X[:, bs1], r128[:])

        b0, outt0 = outts[0]
        nc.sync.dma_start(out_d[0:NBLK, :], outt0[:])
        nc.gpsimd.trigger_dma(count=None)
    nc.finalize()
    return nc


def _get_program() -> bass.Bass:
    if "p" not in _prog_cache:
        _prog_cache["p"] = _build_program()
    return _prog_cache["p"]


def _make_in_maps(encoder_output, attn_W, v):
    f8 = mybir.dt.np(F8)
    w2 = v.astype(np.float64) @ attn_W[:, H:].astype(np.float64)
    w2q = (w2 * W2S).astype(f8)
    w2sb = np.ascontiguousarray(w2q.reshape(KC, 128).T)  # [128, KC]
    constf = np.zeros((128, 148), np.float32)
    constf[:, :128] = np.eye(128, dtype=np.float32)
    constf[:, 128] = 1.0
    cbytes = constf.view(np.uint8).reshape(128, 148 * 4)
    cbytes[:, 516:524] = w2sb.view(np.uint8)
    # cols 132:148 stay zero: int32 ctx indices for the b1 kv_writeback
    enc8 = encoder_output.astype(f8)  # [T, B, H]
    in_maps = []
    for c in range(N_CORES):
        arr = enc8[:, c * B_C : (c + 1) * B_C, :].transpose(1, 2, 0)  # [b, k, t]
        blob = np.empty((128, TOTAL_FREE), f8)
        off = 0
        col = 0
        for cw in CHUNK_COLS:
            tw = cw * 128
            b, i0 = col // NBLK, (col % NBLK) * 128
            sub = arr[b, :, i0 : i0 + tw].reshape(KC, 128, tw)  # [j, p, tt]
            blob[:, off : off + KC * tw] = sub.transpose(1, 0, 2).reshape(
                128, KC * tw
            )
            off += KC * tw
            col += cw
        in_maps.append({"enc": blob, "constf": constf})
    return in_maps


def _assemble(results) -> np.ndarray:
    outs = [r["out"].reshape(B_C, T) for r in results]
    return np.concatenate(outs, axis=0)[:, None, :].astype(np.float32)


def kernel(hidden, encoder_output, attn_W, attn_b, v, **run_kwargs):
    encoder_output = np.asarray(encoder_output, dtype=np.float32)
    attn_W = np.asarray(attn_W, dtype=np.float32)
    v = np.asarray(v, dtype=np.float32)
    in_maps = _make_in_maps(encoder_output, attn_W, v)
    res = run_bass_kernel_spmd(
        _get_program(), in_maps, core_ids=list(range(N_CORES)), **run_kwargs
    )
    out = _assemble(res.results)
    if run_kwargs:
        return out, res
    return out


# revision 18
# speedup vs baseline: 2.1108x; 1.0072x over previous
"""Trainium2 Bass kernel for nn_ConcatAttn.

Reference computes, per batch b:
    energy[t, h] = Linear(2H->H)(concat(hidden[b], enc[t, b]))      # [T, H]
    attn[t]      = energy[t] . v                                    # [T]
    out[b]       = softmax_t(attn)                                  # [T]

Key identity: split the Linear weight W = [W1 | W2] along its input dim.
    attn[t] = (hidden[b] @ W1.T + enc[t,b] @ W2.T + bias) . v
            = enc[t,b] . (v @ W2)  +  const(b)
The const(b) term is constant over t and softmax is shift-invariant, so
    out[b] = softmax_t(enc[:, b] . w2),   w2 = v @ W[:, H:]
i.e. a single matvec against a precomputed 1024-vector, memory-bound on
streaming encoder_output, data-parallel over B across 8 cores.

Design (per core, B_c = 2 batches, T = 2048, H = 1024):
  - enc streams as fp8 e4m3 (4 MiB/core, ~11.7us at the 360 GB/s
    per-core DMA rate; fp8 quantization of enc dominates the error,
    L2 rel err ~9e-3 vs the 2e-2 gate).  w2 is scaled by 128 (power of
    two) before fp8 quantization to stay in e4m3's normal range; the
    scale is divided out inside the ACT exp (scale=1/128).
  - the k-contraction runs on the PE engine: enc arrives
    pre-transposed as [k, t] tiles; each [128k x 128t] tile is the
    stationary operand of a matmul whose moving operand is the
    matching 128-slice of w2 ([128, 1]), accumulating energy columns
    E[:, col] in PSUM over the 8 k-chunks (start at j=0, stop at j=7).
  - per-batch softmax, all in [128, col] orientation: ACT exp with
    accum_out row sums, PE ones-matmul (stride-0 stationary broadcast)
    for the cross-partition total on all 128 partitions, then DVE
    reciprocal + scale back-to-back.  No max-subtraction: |energy|<1.5.
    For batch 1 the last column's exp runs without accum_out (saves the
    187ns accumulator read in the tail); its row sums are folded into
    the total via a second accumulating ones-matmul with X[:,31]
    broadcast as the stationary operand.
  - stores go out via kv_writeback, which consumes [d_head=128,
    batch=16] directly -- no transpose.  Descriptors are pre-generated
    on gpsimd (prepare_only) while the stream runs; trigger_dma fires
    them, skipping the HWDGE gen + DGE latency (~1.3us) a plain
    dma_start would pay.  The baked completion sems must be the tile
    clock's DMASW lane sems (lane rotation: constf L0, prep0 L1,
    prep1 L2) so the exit drain observes the DMAs landing.
  - chunk schedule [4,4,4,4,4,4,4,3,1] columns: batch 0 finishes at
    chunk 3 (its whole softmax+store hides mid-stream), and the final
    chunk carries a single column so only exp(col31) + total + recip +
    scale + trigger sit after the last byte.
"""

import numpy as np
from contextlib import ExitStack

import concourse.bass as bass
import concourse.bacc as bacc
import concourse.mybir as mybir
from concourse import tile
from concourse.bass_utils import run_bass_kernel_spmd

H = 1024
T = 2048
B = 16
N_CORES = 8
B_C = B // N_CORES          # batches per core
NBLK = T // 128             # 128-row tiles per batch
NCOL = B_C * NBLK           # energy columns per core
KC = H // 128               # k-chunks per contraction
W2S = 128.0                 # power-of-2 scale for w2 fp8 quantization
F32 = mybir.dt.float32
F8 = mybir.dt.float8e4

# chunk schedule in columns (128 t each); chunks may not span batches
CHUNK_COLS = [4, 4, 4, 4, 4, 4, 4, 3, 1]
TOTAL_FREE = NCOL * KC * 128
# constf f32 cols: [0] ones, [1:3] w2 fp8 bytes, [3:19] int32 ctx zeros, [19] pad
NCONST = 20

_prog_cache = {}


def _build_program() -> bass.Bass:
    nc = bacc.Bacc("TRN2", target_bir_lowering=False, num_devices=N_CORES)
    enc_d = nc.dram_tensor("enc", [128, TOTAL_FREE], F8, kind="ExternalInput")
    constf_d = nc.dram_tensor("constf", [128, NCONST], F32, kind="ExternalInput")
    out_d = nc.dram_tensor("out", [NCOL, 128], F32, kind="ExternalOutput")

    with ExitStack() as ctx:
        tc = ctx.enter_context(tile.TileContext(nc))
        const_pool = ctx.enter_context(tc.tile_pool(name="const", bufs=1))
        in_pool = ctx.enter_context(tc.tile_pool(name="inp", bufs=1))
        small_pool = ctx.enter_context(tc.tile_pool(name="small", bufs=1))
        psum_pool = ctx.enter_context(tc.tile_pool(name="psum", bufs=1, space="PSUM"))

        # consts in one SWDGE (gpsimd) DMA so they don't serialize ahead of
        # the enc chunk loads in the HWDGE FIFO
        constf = const_pool.tile([128, NCONST], F32, tag="constf")
        nc.gpsimd.dma_start(constf[:], constf_d[:])
        ones = constf[:, 0:1]
        w2sb = constf[:, 1:3].bitcast(F8)
        ctx0 = constf[:, 3:19].bitcast(mybir.dt.int32)

        E = psum_pool.tile([128, NCOL], F32, tag="E")
        X = small_pool.tile([128, NCOL], F32, tag="X")
        S = small_pool.tile([128, B_C], F32, tag="S")
        Xs0 = small_pool.tile([128, NBLK], F32, tag="Xs0")
        Xs1 = small_pool.tile([128, NBLK], F32, tag="Xs1")

        swdge_sems = tc.sems.swdge_block()

        def wb_prep(rows, src, sem):
            nc.gpsimd.kv_writeback(
                out_d[rows * NBLK : (rows + 1) * NBLK, :].rearrange(
                    "b (h a c) -> b h a c", h=128, a=1
                ),
                src[:].rearrange("h (a b c) -> h a b c", a=1, c=1),
                ctx0,
                prepare_only=True,
                sem=sem,
            )

        # b0 store descriptors up front (data dep on Xs0 defers to trigger);
        # lane rotation puts this prep on DMASW lane 1
        wb_prep(0, Xs0, swdge_sems[1])

        def batch_total_recip_scale(b, Xsb, extra_col=None):
            tot_ps = psum_pool.tile([128, 1], F32, tag=f"tot{b}")
            nc.tensor.matmul(
                tot_ps[:],
                lhsT=S[:, b : b + 1].broadcast_to((128, 128)),
                rhs=ones,
                start=True,
                stop=extra_col is None,
            )
            if extra_col is not None:
                nc.tensor.matmul(
                    tot_ps[:],
                    lhsT=X[:, extra_col : extra_col + 1].broadcast_to((128, 128)),
                    rhs=ones,
                    start=False,
                    stop=True,
                )
            r = small_pool.tile([128, 1], F32, tag=f"r{b}")
            nc.vector.reciprocal(r[:], tot_ps[:])
            nc.vector.tensor_scalar_mul(
                Xsb[:], X[:, b * NBLK : (b + 1) * NBLK], r[:]
            )

        off = 0
        col = 0
        for ci, cw in enumerate(CHUNK_COLS):
            tw = cw * 128
            tin = in_pool.tile([128, KC * tw], F8, tag=f"tin{ci}")
            nc.sync.dma_start(tin[:], enc_d[:, off : off + KC * tw])
            for i in range(cw):
                for j in range(KC):
                    nc.tensor.matmul(
                        E[:, col : col + 1],
                        lhsT=tin[:, j * tw + i * 128 : j * tw + (i + 1) * 128],
                        rhs=w2sb[:, j : j + 1],
                        start=(j == 0),
                        stop=(j == KC - 1),
                    )
                col += 1
            off += KC * tw
            if col == NBLK:
                # batch 0 complete: exp + total + recip + scale + fire its
                # writeback; everything hides inside the remaining stream
                nc.scalar.activation(
                    X[:, 0:NBLK],
                    E[:, 0:NBLK],
                    mybir.ActivationFunctionType.Exp,
                    scale=1.0 / W2S,
                    accum_out=S[:, 0:1],
                )
                batch_total_recip_scale(0, Xs0)
                nc.gpsimd.trigger_dma(count=1)
                # b1 store descriptors (on DMASW lane 0, after constf);
                # emitted after trigger#1 so the pending-prep bookkeeping
                # pairs each trigger with its own prep
                wb_prep(1, Xs1, swdge_sems[2])
            elif col == NCOL - 1:
                # all of batch 1 except the final column: exp + row sums
                # while the last chunk streams
                nc.scalar.activation(
                    X[:, NBLK : NCOL - 1],
                    E[:, NBLK : NCOL - 1],
                    mybir.ActivationFunctionType.Exp,
                    scale=1.0 / W2S,
                    accum_out=S[:, 1:2],
                )
        # tail: final column's exp without the accumulator read; its row
        # sums fold into the total via the second accumulating ones-matmul
        nc.scalar.activation(
            X[:, NCOL - 1 : NCOL],
            E[:, NCOL - 1 : NCOL],
            mybir.ActivationFunctionType.Exp,
            scale=1.0 / W2S,
        )
        batch_total_recip_scale(1, Xs1, extra_col=NCOL - 1)
        nc.gpsimd.trigger_dma(count=None)
    nc.finalize()
    return nc


def _get_program() -> bass.Bass:
    if "p" not in _prog_cache:
        _prog_cache["p"] = _build_program()
    return _prog_cache["p"]


def _make_in_maps(encoder_output, attn_W, v):
    f8 = mybir.dt.np(F8)
    w2 = v.astype(np.float64) @ attn_W[:, H:].astype(np.float64)
    w2q = (w2 * W2S).astype(f8)
    w2sb = np.ascontiguousarray(w2q.reshape(KC, 128).T)  # [128, KC]
    constf = np.zeros((128, NCONST), np.float32)
    constf[:, 0] = 1.0
    cbytes = constf.view(np.uint8).reshape(128, NCONST * 4)
    cbytes[:, 4:12] = w2sb.view(np.uint8)
    # cols 3:19 stay zero: int32 ctx indices for the kv_writebacks
    enc8 = encoder_output.astype(f8)  # [T, B, H]
    in_maps = []
    for c in range(N_CORES):
        arr = enc8[:, c * B_C : (c + 1) * B_C, :].transpose(1, 2, 0)  # [b, k, t]
        blob = np.empty((128, TOTAL_FREE), f8)
        off = 0
        col = 0
        for cw in CHUNK_COLS:
            tw = cw * 128
            b, i0 = col // NBLK, (col % NBLK) * 128
            sub = arr[b, :, i0 : i0 + tw].reshape(KC, 128, tw)  # [j, p, tt]
            blob[:, off : off + KC * tw] = sub.transpose(1, 0, 2).reshape(
                128, KC * tw
            )
            off += KC * tw
            col += cw
        in_maps.append({"enc": blob, "constf": constf})
    return in_maps


def _assemble(results) -> np.ndarray:
    outs = [r["out"].reshape(B_C, T) for r in results]
    return np.concatenate(outs, axis=0)[:, None, :].astype(np.float32)


def kernel(hidden, encoder_output, attn_W, attn_b, v, **run_kwargs):
    encoder_output = np.asarray(encoder_output, dtype=np.float32)
    attn_W = np.asarray(attn_W, dtype=np.float32)
    v = np.asarray(v, dtype=np.float32)
    in_maps = _make_in_maps(encoder_output, attn_W, v)
    res = run_bass_kernel_spmd(
        _get_program(), in_maps, core_ids=list(range(N_CORES)), **run_kwargs
    )
    out = _assemble(res.results)
    if run_kwargs:
        return out, res
    return out


# revision 19
# speedup vs baseline: 2.1192x; 1.0040x over previous
"""Trainium2 Bass kernel for nn_ConcatAttn.

Reference computes, per batch b:
    energy[t, h] = Linear(2H->H)(concat(hidden[b], enc[t, b]))      # [T, H]
    attn[t]      = energy[t] . v                                    # [T]
    out[b]       = softmax_t(attn)                                  # [T]

Key identity: split the Linear weight W = [W1 | W2] along its input dim.
    attn[t] = (hidden[b] @ W1.T + enc[t,b] @ W2.T + bias) . v
            = enc[t,b] . (v @ W2)  +  const(b)
The const(b) term is constant over t and softmax is shift-invariant, so
    out[b] = softmax_t(enc[:, b] . w2),   w2 = v @ W[:, H:]
i.e. a single matvec against a precomputed 1024-vector, memory-bound on
streaming encoder_output, data-parallel over B across 8 cores.

Design (per core, B_c = 2 batches, T = 2048, H = 1024):
  - enc streams as fp8 e4m3 (4 MiB/core, ~11.7us at the 360 GB/s
    per-core DMA rate; fp8 quantization of enc dominates the error,
    L2 rel err ~9e-3 vs the 2e-2 gate).  w2 is scaled by 128 (power of
    two) before fp8 quantization to stay in e4m3's normal range; the
    scale is divided out inside the ACT exp (scale=1/128).
  - the k-contraction runs on the PE engine: enc arrives
    pre-transposed as [k, t] tiles; each [128k x 128t] tile is the
    stationary operand of a matmul whose moving operand is the
    matching 128-slice of w2 ([128, 1]), accumulating energy columns
    E[:, col] in PSUM over the 8 k-chunks (start at j=0, stop at j=7).
  - per-batch softmax, all in [128, col] orientation: ACT exp with
    accum_out row sums, PE ones-matmul (stride-0 stationary broadcast)
    for the cross-partition total on all 128 partitions, then DVE
    reciprocal + scale back-to-back.  No max-subtraction: |energy|<1.5.
    For batch 1 the last column's exp runs without accum_out (saves the
    187ns accumulator read in the tail); its row sums are folded into
    the total via a second accumulating ones-matmul with X[:,31]
    broadcast as the stationary operand.
  - stores go out via kv_writeback, which consumes [d_head=128,
    batch=16] directly -- no transpose.  Descriptors are pre-generated
    on gpsimd (prepare_only) while the stream runs; trigger_dma fires
    them, skipping the HWDGE gen + DGE latency (~1.3us) a plain
    dma_start would pay.  The baked completion sems must be the tile
    clock's DMASW lane sems (lane rotation: constf L0, prep0 L1,
    prep1 L2) so the exit drain observes the DMAs landing.
  - chunk schedule [4,4,4,4,4,4,4,3,1] columns: batch 0 finishes at
    chunk 3 (its whole softmax+store hides mid-stream), and the final
    chunk carries a single column so only exp(col31) + total + recip +
    scale + trigger sit after the last byte.
"""

import numpy as np
from contextlib import ExitStack

import concourse.bass as bass
import concourse.bacc as bacc
import concourse.mybir as mybir
from concourse import tile
from concourse.bass_utils import run_bass_kernel_spmd

H = 1024
T = 2048
B = 16
N_CORES = 8
B_C = B // N_CORES          # batches per core
NBLK = T // 128             # 128-row tiles per batch
NCOL = B_C * NBLK           # energy columns per core
KC = H // 128               # k-chunks per contraction
W2S = 128.0                 # power-of-2 scale for w2 fp8 quantization
F32 = mybir.dt.float32
F8 = mybir.dt.float8e4

# chunk schedule in columns (128 t each); chunks may not span batches
CHUNK_COLS = [4, 4, 4, 4, 4, 4, 4, 2, 2]
TOTAL_FREE = NCOL * KC * 128
# constf f32 cols: [0] ones, [1:3] w2 fp8 bytes, [3:19] int32 ctx zeros, [19] pad
NCONST = 20

_prog_cache = {}


def _build_program() -> bass.Bass:
    nc = bacc.Bacc("TRN2", target_bir_lowering=False, num_devices=N_CORES)
    enc_d = nc.dram_tensor("enc", [128, TOTAL_FREE], F8, kind="ExternalInput")
    constf_d = nc.dram_tensor("constf", [128, NCONST], F32, kind="ExternalInput")
    out_d = nc.dram_tensor("out", [NCOL, 128], F32, kind="ExternalOutput")

    with ExitStack() as ctx:
        tc = ctx.enter_context(tile.TileContext(nc))
        const_pool = ctx.enter_context(tc.tile_pool(name="const", bufs=1))
        in_pool = ctx.enter_context(tc.tile_pool(name="inp", bufs=1))
        small_pool = ctx.enter_context(tc.tile_pool(name="small", bufs=1))
        psum_pool = ctx.enter_context(tc.tile_pool(name="psum", bufs=1, space="PSUM"))

        # consts in one SWDGE (gpsimd) DMA so they don't serialize ahead of
        # the enc chunk loads in the HWDGE FIFO
        constf = const_pool.tile([128, NCONST], F32, tag="constf")
        nc.gpsimd.dma_start(constf[:], constf_d[:])
        ones = constf[:, 0:1]
        w2sb = constf[:, 1:3].bitcast(F8)
        ctx0 = constf[:, 3:19].bitcast(mybir.dt.int32)

        # energy in three PSUM tiles so the tail-column matmul writes don't
        # pick up a tile-granular WAR hazard against the earlier exps' reads
        E0 = psum_pool.tile([128, NBLK], F32, tag="E0")
        E1a = psum_pool.tile([128, NBLK - 2], F32, tag="E1a")
        E1b = psum_pool.tile([128, 2], F32, tag="E1b")
        X = small_pool.tile([128, NCOL], F32, tag="X")
        S = small_pool.tile([128, B_C], F32, tag="S")
        Xs0 = small_pool.tile([128, NBLK], F32, tag="Xs0")
        Xs1 = small_pool.tile([128, NBLK], F32, tag="Xs1")

        swdge_sems = tc.sems.swdge_block()

        def wb_prep(rows, src, sem):
            nc.gpsimd.kv_writeback(
                out_d[rows * NBLK : (rows + 1) * NBLK, :].rearrange(
                    "b (h a c) -> b h a c", h=128, a=1
                ),
                src[:].rearrange("h (a b c) -> h a b c", a=1, c=1),
                ctx0,
                prepare_only=True,
                sem=sem,
            )

        # b0 store descriptors up front (data dep on Xs0 defers to trigger);
        # lane rotation puts this prep on DMASW lane 1
        wb_prep(0, Xs0, swdge_sems[1])

        def batch_total_recip_scale(b, Xsb, extra_col=None):
            tot_ps = psum_pool.tile([128, 1], F32, tag=f"tot{b}")
            nc.tensor.matmul(
                tot_ps[:],
                lhsT=S[:, b : b + 1].broadcast_to((128, 128)),
                rhs=ones,
                start=True,
                stop=extra_col is None,
            )
            if extra_col is not None:
                for k, c in enumerate(range(extra_col, NCOL)):
                    nc.tensor.matmul(
                        tot_ps[:],
                        lhsT=X[:, c : c + 1].broadcast_to((128, 128)),
                        rhs=ones,
                        start=False,
                        stop=(c == NCOL - 1),
                    )
            r = small_pool.tile([128, 1], F32, tag=f"r{b}")
            nc.vector.reciprocal(r[:], tot_ps[:])
            nc.vector.tensor_scalar_mul(
                Xsb[:], X[:, b * NBLK : (b + 1) * NBLK], r[:]
            )

        off = 0
        col = 0
        for ci, cw in enumerate(CHUNK_COLS):
            tw = cw * 128
            tin = in_pool.tile([128, KC * tw], F8, tag=f"tin{ci}")
            nc.sync.dma_start(tin[:], enc_d[:, off : off + KC * tw])
            for i in range(cw):
                if col < NBLK:
                    ecol = E0[:, col : col + 1]
                elif col < NCOL - 2:
                    ecol = E1a[:, col - NBLK : col - NBLK + 1]
                else:
                    ecol = E1b[:, col - (NCOL - 2) : col - (NCOL - 2) + 1]
                for j in range(KC):
                    nc.tensor.matmul(
                        ecol,
                        lhsT=tin[:, j * tw + i * 128 : j * tw + (i + 1) * 128],
                        rhs=w2sb[:, j : j + 1],
                        start=(j == 0),
                        stop=(j == KC - 1),
                    )
                col += 1
            off += KC * tw
            if col == NBLK:
                # batch 0 complete: exp + total + recip + scale + fire its
                # writeback; everything hides inside the remaining stream
                nc.scalar.activation(
                    X[:, 0:NBLK],
                    E0[:],
                    mybir.ActivationFunctionType.Exp,
                    scale=1.0 / W2S,
                    accum_out=S[:, 0:1],
                )
                batch_total_recip_scale(0, Xs0)
                nc.gpsimd.trigger_dma(count=1)
                # b1 store descriptors (on DMASW lane 0, after constf);
                # emitted after trigger#1 so the pending-prep bookkeeping
                # pairs each trigger with its own prep
                wb_prep(1, Xs1, swdge_sems[2])
            elif col == NCOL - 2:
                # all of batch 1 except the final two columns: exp + row
                # sums while the last chunk streams
                nc.scalar.activation(
                    X[:, NBLK : NCOL - 2],
                    E1a[:],
                    mybir.ActivationFunctionType.Exp,
                    scale=1.0 / W2S,
                    accum_out=S[:, 1:2],
                )
        # tail: final columns' exp without the accumulator read; their row
        # sums fold into the total via accumulating broadcast ones-matmuls
        nc.scalar.activation(
            X[:, NCOL - 2 : NCOL],
            E1b[:],
            mybir.ActivationFunctionType.Exp,
            scale=1.0 / W2S,
        )
        batch_total_recip_scale(1, Xs1, extra_col=NCOL - 2)
        nc.gpsimd.trigger_dma(count=None)
    nc.finalize()
    return nc


def _get_program() -> bass.Bass:
    if "p" not in _prog_cache:
        _prog_cache["p"] = _build_program()
    return _prog_cache["p"]


def _make_in_maps(encoder_output, attn_W, v):
    f8 = mybir.dt.np(F8)
    w2 = v.astype(np.float64) @ attn_W[:, H:].astype(np.float64)
    w2q = (w2 * W2S).astype(f8)
    w2sb = np.ascontiguousarray(w2q.reshape(KC, 128).T)  # [128, KC]
    constf = np.zeros((128, NCONST), np.float32)
    constf[:, 0] = 1.0
    cbytes = constf.view(np.uint8).reshape(128, NCONST * 4)
    cbytes[:, 4:12] = w2sb.view(np.uint8)
    # cols 3:19 stay zero: int32 ctx indices for the kv_writebacks
    enc8 = encoder_output.astype(f8)  # [T, B, H]
    in_maps = []
    for c in range(N_CORES):
        arr = enc8[:, c * B_C : (c + 1) * B_C, :].transpose(1, 2, 0)  # [b, k, t]
        blob = np.empty((128, TOTAL_FREE), f8)
        off = 0
        col = 0
        for cw in CHUNK_COLS:
            tw = cw * 128
            b, i0 = col // NBLK, (col % NBLK) * 128
            sub = arr[b, :, i0 : i0 + tw].reshape(KC, 128, tw)  # [j, p, tt]
            blob[:, off : off + KC * tw] = sub.transpose(1, 0, 2).reshape(
                128, KC * tw
            )
            off += KC * tw
            col += cw
        in_maps.append({"enc": blob, "constf": constf})
    return in_maps


def _assemble(results) -> np.ndarray:
    outs = [r["out"].reshape(B_C, T) for r in results]
    return np.concatenate(outs, axis=0)[:, None, :].astype(np.float32)


def kernel(hidden, encoder_output, attn_W, attn_b, v, **run_kwargs):
    encoder_output = np.asarray(encoder_output, dtype=np.float32)
    attn_W = np.asarray(attn_W, dtype=np.float32)
    v = np.asarray(v, dtype=np.float32)
    in_maps = _make_in_maps(encoder_output, attn_W, v)
    res = run_bass_kernel_spmd(
        _get_program(), in_maps, core_ids=list(range(N_CORES)), **run_kwargs
    )
    out = _assemble(res.results)
    if run_kwargs:
        return out, res
    return out
